# revision 5
# baseline (speedup 1.0000x reference)
"""DISK keypoint detection (NMS + top-k + descriptor gather/normalize) on 8 trn2 cores.

Strategy (per sharding hint): split H across 8 cores (128 rows each).
Each core, in one Bass/Tile NEFF:
  - loads its heatmap rows (+window//2 halo, zero-padded outside the image),
  - computes the 5x5 max-pool via 4 y-shifted tensor_max + 4 x-shifted
    tensor_max ops (separable max filter),
  - builds masked scores: heat where (heat == pooled and heat > 0), else 0
    (0 is a valid sentinel because every real candidate is > 0),
  - extracts the per-image-row top-32 candidates (values + x indices) with
    4 rounds of DVE max8 / max_index / match_replace,
  - gathers the 128-dim descriptors for all 128*32 candidates from its
    pixel-major descriptor shard with one indirect DMA,
  - L2-normalizes them on-chip,
  - writes scores, pixel indices, and normalized descriptors.
Host merges 8*4096 candidates (score > 0) into the global top-K by
(-score, index) — exactly jax.lax.top_k's ordering — and assembles outputs.

Per-row top-32 provably contains each row's contribution to the global
top-2048 unless some single image row holds >32 of the global top-2048
(expected count is 2 per row; P(fail) ~ 1e-23 for iid scores).
"""

import math

import numpy as np

_CACHE: dict = {}

H = 1024
W = 1024
D = 128
M = 8  # cores
ROWS = H // M  # 128


def _build_module(window: int, topc: int):
    import concourse.bacc as bacc
    import concourse.bass as bass
    import concourse.mybir as mybir
    from concourse.tile import TileContext

    fp32 = mybir.dt.float32
    u32 = mybir.dt.uint32
    halo = window // 2

    nc = bacc.Bacc(trn_type="TRN2", debug=False, num_devices=M)

    heat_in = nc.dram_tensor(
        "heat_halo", [ROWS + 2 * halo, W], fp32, kind="ExternalInput"
    )
    desc_in = nc.dram_tensor("desc_shard", [ROWS * W, D], fp32, kind="ExternalInput")
    vals_out = nc.dram_tensor("vals", [ROWS, topc], fp32, kind="ExternalOutput")
    pix_out = nc.dram_tensor("pix", [ROWS, topc], u32, kind="ExternalOutput")
    desc_out = nc.dram_tensor("descs", [ROWS, topc * D], fp32, kind="ExternalOutput")

    with TileContext(nc) as tc:
        with tc.tile_pool(name="pool", bufs=1) as pool:
            # --- load y-shifted heatmap windows (single DMA, overlapping
            # source reads: sall[p, dy, :] = heat_halo[p + dy, :]) ---
            sall = pool.tile([ROWS, window * W], fp32, tag="sall", name="sall")
            src = bass.AP(heat_in.ap().tensor, 0, [[W, ROWS], [W, window], [1, W]])
            nc.sync.dma_start(
                sall[:, :].rearrange("p (k x) -> p k x", k=window), src
            )
            sv = sall[:, :].rearrange("p (k x) -> p k x", k=window)
            s = [sv[:, dy, :] for dy in range(window)]
            heat = s[halo]  # the un-shifted rows

            # --- y-direction max (tree) ---
            a = pool.tile([ROWS, W], fp32, tag="a", name="a")
            b = pool.tile([ROWS, W], fp32, tag="b", name="b")
            nc.vector.tensor_max(a[:, :], s[0], s[1])
            nc.vector.tensor_max(b[:, :], s[2], s[3])
            nc.vector.tensor_max(a[:, :], a[:, :], b[:, :])
            # P: x-padded y-max, pad cols = 0 sentinel
            P = pool.tile([ROWS, W + 2 * halo * 2], fp32, tag="P", name="P")
            nc.gpsimd.memset(P[:, 0 : 2 * halo], 0.0)
            nc.gpsimd.memset(P[:, W + 2 * halo : W + 4 * halo], 0.0)
            nc.vector.tensor_max(P[:, 2 * halo : W + 2 * halo], a[:, :], s[4][:, :])

            # --- x-direction max (tree over shifted views) ---
            q1 = a
            q2 = b
            nc.vector.tensor_max(q1[:, :], P[:, 0:W], P[:, 1 : W + 1])
            nc.vector.tensor_max(q2[:, :], P[:, 2 : W + 2], P[:, 3 : W + 3])
            nc.vector.tensor_max(q1[:, :], q1[:, :], q2[:, :])
            pooled = pool.tile([ROWS, W], fp32, tag="pooled", name="pooled")
            nc.vector.tensor_max(pooled[:, :], q1[:, :], P[:, 4 : W + 4])

            # --- masked scores: heat if (heat>0 and heat==pooled) else 0 ---
            heatpos = q2
            nc.vector.scalar_tensor_tensor(
                heatpos[:, :],
                heat[:, :],
                0.0,
                heat[:, :],
                op0=mybir.AluOpType.is_gt,
                op1=mybir.AluOpType.mult,
            )
            cnd = q1
            nc.vector.tensor_tensor(
                cnd[:, :], heat[:, :], pooled[:, :], op=mybir.AluOpType.is_ge
            )
            masked = pool.tile([ROWS, W], fp32, tag="masked", name="masked")
            nc.vector.tensor_mul(masked[:, :], cnd[:, :], heatpos[:, :])

            # --- per-row top-`topc` extraction ---
            vals = pool.tile([ROWS, topc], fp32, tag="vals", name="vals")
            xidx = pool.tile([ROWS, topc], u32, tag="xidx", name="xidx")
            for it in range(topc // 8):
                sl = slice(8 * it, 8 * it + 8)
                nc.vector.max(out=vals[:, sl], in_=masked[:, :])
                nc.vector.max_index(
                    out=xidx[:, sl], in_max=vals[:, sl], in_values=masked[:, :]
                )
                if it != topc // 8 - 1:
                    nc.vector.match_replace(
                        out=masked[:, :],
                        in_to_replace=vals[:, sl],
                        in_values=masked[:, :],
                        imm_value=0.0,
                    )

            # --- pixel indices within shard: p*W + x ---
            pix = pool.tile([ROWS, topc], u32, tag="pix", name="pix")
            nc.gpsimd.iota(
                pix[:, :], pattern=[[0, topc]], base=0, channel_multiplier=W
            )
            nc.gpsimd.tensor_tensor(
                pix[:, :], pix[:, :], xidx[:, :], op=mybir.AluOpType.add
            )

            # --- descriptor gather: dsc[p, c, :] = desc_shard[pix[p,c], :] ---
            dsc = pool.tile([ROWS, topc * D], fp32, tag="dsc", name="dsc")
            dsc3 = dsc[:, :].rearrange("p (c d) -> p c d", d=D)
            nc.gpsimd.indirect_dma_start(
                out=dsc3,
                out_offset=None,
                in_=desc_in.ap(),
                in_offset=bass.IndirectOffsetOnAxis(ap=pix[:, :], axis=0),
            )

            # --- L2 normalize: d / max(sqrt(sum d^2), 1e-12) ---
            sq = pool.tile([ROWS, topc * D], fp32, tag="sq", name="sq")
            nc.scalar.activation(
                sq[:, :], dsc[:, :], mybir.ActivationFunctionType.Square
            )
            nrm = pool.tile([ROWS, topc], fp32, tag="nrm", name="nrm")
            nc.vector.reduce_sum(
                out=nrm[:, :],
                in_=sq[:, :].rearrange("p (c d) -> p c d", d=D),
                axis=mybir.AxisListType.X,
            )
            nc.scalar.activation(
                nrm[:, :], nrm[:, :], mybir.ActivationFunctionType.Sqrt
            )
            nc.vector.tensor_scalar_max(nrm[:, :], nrm[:, :], 1e-12)
            inv = pool.tile([ROWS, topc], fp32, tag="inv", name="inv")
            nc.vector.reciprocal(inv[:, :], nrm[:, :])

            inv_ap = inv[:, :]
            inv3 = bass.AP(inv_ap.tensor, inv_ap.offset, list(inv_ap.ap) + [[0, D]])
            dn = sq  # reuse
            nc.vector.tensor_tensor(
                dn[:, :].rearrange("p (c d) -> p c d", d=D),
                dsc3,
                inv3,
                op=mybir.AluOpType.mult,
            )

            # --- outputs ---
            nc.sync.dma_start(vals_out.ap(), vals[:, :])
            nc.sync.dma_start(pix_out.ap(), pix[:, :])
            nc.sync.dma_start(desc_out.ap(), dn[:, :])

    nc.compile()
    return nc


def _transpose_desc(desc_flat: np.ndarray) -> np.ndarray:
    """[D, H*W] -> [H*W, D] contiguous, cache-blocked."""
    d, n = desc_flat.shape
    out = np.empty((n, d), dtype=desc_flat.dtype)
    blk = 8192
    for i in range(0, n, blk):
        out[i : i + blk] = desc_flat[:, i : i + blk].T
    return out


def make_in_maps(features: np.ndarray, window: int):
    """Shard full features into per-core input maps."""
    halo = window // 2
    heat = np.ascontiguousarray(features[0, D], dtype=np.float32)  # [H, W]
    desc_t = _transpose_desc(features[0, :D].reshape(D, H * W))  # [H*W, D]

    in_maps = []
    for c in range(M):
        r0 = c * ROWS
        hh = np.zeros((ROWS + 2 * halo, W), np.float32)
        lo = max(0, r0 - halo)
        hi = min(H, r0 + ROWS + halo)
        hh[lo - (r0 - halo) : hi - (r0 - halo), :] = heat[lo:hi]
        in_maps.append(
            {
                "heat_halo": hh,
                "desc_shard": desc_t[r0 * W : (r0 + ROWS) * W],
            }
        )
    return in_maps


def merge_outputs(results, orig_w, orig_h, k):
    """Host-side global top-k merge of per-core candidates."""
    vals = np.stack([np.asarray(r["vals"]) for r in results])  # [M,ROWS,topc]
    pix = np.stack([np.asarray(r["pix"]) for r in results])  # [M,ROWS,topc]
    dsc = np.stack([np.asarray(r["descs"]) for r in results])  # [M,ROWS,topc*D]
    topc = vals.shape[2]

    scores = vals.reshape(-1)
    gidx = (
        pix.astype(np.int64) + (np.arange(M, dtype=np.int64) * (ROWS * W))[:, None, None]
    ).reshape(-1)
    dsc = dsc.reshape(M, ROWS, topc, D).reshape(-1, D)

    x = gidx % W
    y = gidx // W
    keep = scores > 0.0
    if orig_w < W or orig_h < H:
        keep &= (x <= orig_w - 1) & (y <= orig_h - 1)
    cand = np.flatnonzero(keep)
    if cand.size < k:
        raise RuntimeError(
            f"only {cand.size} NMS candidates found, need {k}; "
            "per-row candidate budget exceeded"
        )
    order = np.lexsort((gidx[cand], -scores[cand]))
    sel = cand[order[:k]]

    keypoints = np.stack([x[sel], y[sel]], axis=-1).astype(np.float32)
    sel_scores = scores[sel].astype(np.float32)
    d = np.ascontiguousarray(dsc[sel], dtype=np.float32)
    valid = np.ones(k, dtype=bool)
    return keypoints, sel_scores, d, valid


def kernel(features, orig_w, orig_h, max_keypoints, window):
    from concourse.bass_utils import run_bass_kernel_spmd

    features = np.asarray(features, dtype=np.float32)
    orig_w = int(orig_w)
    orig_h = int(orig_h)
    k = int(max_keypoints)
    window = int(window)

    assert features.shape == (1, D + 1, H, W), features.shape
    assert window == 5, f"kernel specialized for window=5, got {window}"

    # per-image-row candidate budget; 32 covers k=2048 with huge margin
    topc = min(64, max(32, 8 * math.ceil(4 * k / H / 8)))

    key = (window, topc)
    if key not in _CACHE:
        _CACHE[key] = _build_module(window, topc)
    nc = _CACHE[key]

    in_maps = make_in_maps(features, window)
    res = run_bass_kernel_spmd(nc, in_maps, core_ids=list(range(M)))
    return merge_outputs(res.results, orig_w, orig_h, k)


# revision 8
# speedup vs baseline: 2.2939x; 2.2939x over previous
"""DISK keypoint detection (NMS + top-k + descriptor gather/normalize) on 8 trn2 cores.

Sharding: H split across 8 cores (128 rows each). Per core, one Bass/Tile NEFF:
  - load heatmap rows + window//2 halo (zero-padded outside the image) as 5
    y-shifted copies via two overlapping-AP DMAs (left/right column halves),
  - separable 5x5 max-pool: y-max tree per half, then x-max tree over an
    x-padded tile,
  - masked scores: heat where (heat > 0 and heat >= pooled) else 0
    (0 sentinel is exact: every real candidate is > 0),
  - per-image-row top-16 via two rounds of DVE max8/find_index8(/match_replace),
  - descriptor gather: 16 indirect DMAs in [128 partitions, 1 offset] form
    (the only HW-correct shape), one per candidate rank, pipelined after each
    extraction round,
  - L2-normalize on-chip in two 8-rank chunks,
  - outputs: scores [128,16], pixel indices [128,16], descriptors [128,16*128].
Host merges 8*2048 candidates (score > 0) by (-score, index) — jax.lax.top_k
order — into the global top-K and assembles outputs.

Coverage: global top-2048 is contained in per-row top-16 unless one image row
holds >16 of the global top-2048 (expected 2 per row; P ~ 5e-8 for iid scores).
"""

import math

import numpy as np

_CACHE: dict = {}

H = 1024
W = 1024
D = 128
M = 8  # cores
ROWS = H // M  # 128
TOPC = 16  # per-row candidates kept = gather calls


def _build_module(window: int):
    import concourse.bacc as bacc
    import concourse.bass as bass
    import concourse.mybir as mybir
    from concourse.tile import TileContext

    fp32 = mybir.dt.float32
    u32 = mybir.dt.uint32
    halo = window // 2  # 2
    assert window == 5

    nc = bacc.Bacc(trn_type="TRN2", debug=False, num_devices=M)

    heat_in = nc.dram_tensor(
        "heat_halo", [ROWS + 2 * halo, W], fp32, kind="ExternalInput"
    )
    desc_in = nc.dram_tensor("desc_shard", [ROWS * W, D], fp32, kind="ExternalInput")
    vals_out = nc.dram_tensor("vals", [ROWS, TOPC], fp32, kind="ExternalOutput")
    pix_out = nc.dram_tensor("pix", [ROWS, TOPC], u32, kind="ExternalOutput")
    desc_out = nc.dram_tensor("descs", [ROWS, TOPC * D], fp32, kind="ExternalOutput")

    # column split: half 0 covers x [0, XS), half 1 covers [XS, W)
    XS = 516  # half-0 y-max width (x 0..516), half 1 = 508 cols
    W0, W1 = XS, W - XS

    with TileContext(nc) as tc:
        with tc.tile_pool(name="pool", bufs=1) as pool:
            # --- loads: sall_h[p, k, i] = heat_halo[p + k, x0 + i] ---
            hsrc = heat_in.ap().tensor
            sall0 = pool.tile([ROWS, window * W0], fp32, tag="s0", name="sall0")
            src0 = bass.AP(hsrc, 0, [[W, ROWS], [W, window], [1, W0]])
            nc.sync.dma_start(
                sall0[:, :].rearrange("p (k x) -> p k x", k=window), src0
            )
            sall1 = pool.tile([ROWS, window * W1], fp32, tag="s1", name="sall1")
            src1 = bass.AP(hsrc, XS, [[W, ROWS], [W, window], [1, W1]])
            nc.sync.dma_start(
                sall1[:, :].rearrange("p (k x) -> p k x", k=window), src1
            )
            sv0 = sall0[:, :].rearrange("p (k x) -> p k x", k=window)
            sv1 = sall1[:, :].rearrange("p (k x) -> p k x", k=window)

            # --- y-max trees into x-padded P (pad cols are 0) ---
            P = pool.tile([ROWS, W + 2 * halo], fp32, tag="P", name="P")
            nc.gpsimd.memset(P[:, 0:halo], 0.0)
            nc.gpsimd.memset(P[:, W + halo : W + 2 * halo], 0.0)
            a0 = pool.tile([ROWS, W0], fp32, tag="a0", name="a0")
            b0 = pool.tile([ROWS, W0], fp32, tag="b0", name="b0")
            nc.vector.tensor_max(a0[:, :], sv0[:, 0, :], sv0[:, 1, :])
            nc.vector.tensor_max(b0[:, :], sv0[:, 2, :], sv0[:, 3, :])
            nc.vector.tensor_max(a0[:, :], a0[:, :], b0[:, :])
            nc.vector.tensor_max(P[:, 2 : 2 + W0], a0[:, :], sv0[:, 4, :])
            a1 = pool.tile([ROWS, W1], fp32, tag="a1", name="a1")
            b1 = pool.tile([ROWS, W1], fp32, tag="b1", name="b1")
            nc.vector.tensor_max(a1[:, :], sv1[:, 0, :], sv1[:, 1, :])
            nc.vector.tensor_max(b1[:, :], sv1[:, 2, :], sv1[:, 3, :])
            nc.vector.tensor_max(a1[:, :], a1[:, :], b1[:, :])
            nc.vector.tensor_max(P[:, 2 + W0 : 2 + W], a1[:, :], sv1[:, 4, :])

            # --- x-max tree ---
            q1 = pool.tile([ROWS, W], fp32, tag="q1", name="q1")
            q2 = pool.tile([ROWS, W], fp32, tag="q2", name="q2")
            nc.vector.tensor_max(q1[:, :], P[:, 0:W], P[:, 1 : W + 1])
            nc.vector.tensor_max(q2[:, :], P[:, 2 : W + 2], P[:, 3 : W + 3])
            nc.vector.tensor_max(q1[:, :], q1[:, :], q2[:, :])
            pooled = q2
            nc.vector.tensor_max(pooled[:, :], q1[:, :], P[:, 4 : W + 4])

            # --- masked scores: (heat > 0) * (heat >= pooled) * heat ---
            # relu(heat) on the scalar engine, mask+mult on DVE, per half
            hp = pool.tile([ROWS, W], fp32, tag="hp", name="hp")
            nc.scalar.activation(
                hp[:, 0:W0], sv0[:, halo, :], mybir.ActivationFunctionType.Relu
            )
            nc.scalar.activation(
                hp[:, W0:W], sv1[:, halo, :], mybir.ActivationFunctionType.Relu
            )
            cnd = pool.tile([ROWS, W], fp32, tag="cnd", name="cnd")
            nc.vector.tensor_tensor(
                cnd[:, 0:W0], sv0[:, halo, :], pooled[:, 0:W0],
                op=mybir.AluOpType.is_ge,
            )
            nc.vector.tensor_tensor(
                cnd[:, W0:W], sv1[:, halo, :], pooled[:, W0:W],
                op=mybir.AluOpType.is_ge,
            )
            masked = pool.tile([ROWS, W], fp32, tag="masked", name="masked")
            nc.vector.tensor_mul(masked[:, :], cnd[:, :], hp[:, :])

            # --- per-row top-16: 2 rounds of max8/find_index8/match_replace ---
            vals = pool.tile([ROWS, TOPC], fp32, tag="vals", name="vals")
            xidx = pool.tile([ROWS, TOPC], u32, tag="xidx", name="xidx")
            rowbase = pool.tile([ROWS, TOPC], u32, tag="rowbase", name="rowbase")
            nc.gpsimd.iota(
                rowbase[:, :], pattern=[[0, TOPC]], base=0, channel_multiplier=W
            )
            pix = pool.tile([ROWS, TOPC], u32, tag="pix", name="pix")
            dscs = []
            for r in range(TOPC // 8):
                sl = slice(8 * r, 8 * r + 8)
                nc.vector.max(out=vals[:, sl], in_=masked[:, :])
                nc.vector.max_index(
                    out=xidx[:, sl], in_max=vals[:, sl], in_values=masked[:, :]
                )
                if r != TOPC // 8 - 1:
                    nc.vector.match_replace(
                        out=masked[:, :],
                        in_to_replace=vals[:, sl],
                        in_values=masked[:, :],
                        imm_value=0.0,
                    )
                # pixel indices for this round: p*W + x
                nc.gpsimd.tensor_tensor(
                    pix[:, sl], rowbase[:, sl], xidx[:, sl], op=mybir.AluOpType.add
                )
                # descriptor gathers for this round's 8 ranks
                dsc = pool.tile([ROWS, 8 * D], fp32, tag=f"dsc{r}", name=f"dsc{r}")
                for c in range(8):
                    nc.gpsimd.indirect_dma_start(
                        out=dsc[:, c * D : (c + 1) * D],
                        out_offset=None,
                        in_=desc_in.ap(),
                        in_offset=bass.IndirectOffsetOnAxis(
                            ap=pix[:, 8 * r + c : 8 * r + c + 1], axis=0
                        ),
                    )
                dscs.append(dsc)

            # --- L2 normalize per 8-rank chunk ---
            for r, dsc in enumerate(dscs):
                sq = pool.tile([ROWS, 8 * D], fp32, tag=f"sq{r}", name=f"sq{r}")
                nc.scalar.activation(
                    sq[:, :], dsc[:, :], mybir.ActivationFunctionType.Square
                )
                nrm = pool.tile([ROWS, 8], fp32, tag=f"nrm{r}", name=f"nrm{r}")
                nc.vector.reduce_sum(
                    out=nrm[:, :],
                    in_=sq[:, :].rearrange("p (c d) -> p c d", d=D),
                    axis=mybir.AxisListType.X,
                )
                nc.scalar.activation(
                    nrm[:, :], nrm[:, :], mybir.ActivationFunctionType.Sqrt
                )
                nc.vector.tensor_scalar_max(nrm[:, :], nrm[:, :], 1e-12)
                inv = pool.tile([ROWS, 8], fp32, tag=f"inv{r}", name=f"inv{r}")
                nc.vector.reciprocal(inv[:, :], nrm[:, :])
                inv_ap = inv[:, :]
                inv3 = bass.AP(
                    inv_ap.tensor, inv_ap.offset, list(inv_ap.ap) + [[0, D]]
                )
                dn = sq  # reuse
                nc.vector.tensor_tensor(
                    dn[:, :].rearrange("p (c d) -> p c d", d=D),
                    dsc[:, :].rearrange("p (c d) -> p c d", d=D),
                    inv3,
                    op=mybir.AluOpType.mult,
                )
                nc.sync.dma_start(
                    desc_out.ap()[:, r * 8 * D : (r + 1) * 8 * D], dn[:, :]
                )

            # --- small outputs ---
            nc.sync.dma_start(vals_out.ap(), vals[:, :])
            nc.sync.dma_start(pix_out.ap(), pix[:, :])

    nc.compile()
    return nc


def _transpose_desc(desc_flat: np.ndarray) -> np.ndarray:
    """[D, H*W] -> [H*W, D] contiguous, cache-blocked."""
    d, n = desc_flat.shape
    out = np.empty((n, d), dtype=desc_flat.dtype)
    blk = 8192
    for i in range(0, n, blk):
        out[i : i + blk] = desc_flat[:, i : i + blk].T
    return out


def make_in_maps(features: np.ndarray, window: int):
    """Shard full features into per-core input maps."""
    halo = window // 2
    heat = np.ascontiguousarray(features[0, D], dtype=np.float32)  # [H, W]
    desc_t = _transpose_desc(features[0, :D].reshape(D, H * W))  # [H*W, D]

    in_maps = []
    for c in range(M):
        r0 = c * ROWS
        hh = np.zeros((ROWS + 2 * halo, W), np.float32)
        lo = max(0, r0 - halo)
        hi = min(H, r0 + ROWS + halo)
        hh[lo - (r0 - halo) : hi - (r0 - halo), :] = heat[lo:hi]
        in_maps.append(
            {
                "heat_halo": hh,
                "desc_shard": desc_t[r0 * W : (r0 + ROWS) * W],
            }
        )
    return in_maps


def merge_outputs(results, orig_w, orig_h, k):
    """Host-side global top-k merge of per-core candidates."""
    vals = np.stack([np.asarray(r["vals"]) for r in results])  # [M,ROWS,TOPC]
    pix = np.stack([np.asarray(r["pix"]) for r in results])
    dsc = np.stack([np.asarray(r["descs"]) for r in results])
    topc = vals.shape[2]

    scores = vals.reshape(-1)
    gidx = (
        pix.astype(np.int64) + (np.arange(M, dtype=np.int64) * (ROWS * W))[:, None, None]
    ).reshape(-1)
    dsc = dsc.reshape(M, ROWS, topc, D).reshape(-1, D)

    x = gidx % W
    y = gidx // W
    keep = scores > 0.0
    if orig_w < W or orig_h < H:
        keep &= (x <= orig_w - 1) & (y <= orig_h - 1)
    cand = np.flatnonzero(keep)
    if cand.size < k:
        raise RuntimeError(
            f"only {cand.size} NMS candidates found, need {k}; "
            "per-row candidate budget exceeded"
        )
    order = np.lexsort((gidx[cand], -scores[cand]))
    sel = cand[order[:k]]

    keypoints = np.stack([x[sel], y[sel]], axis=-1).astype(np.float32)
    sel_scores = scores[sel].astype(np.float32)
    d = np.ascontiguousarray(dsc[sel], dtype=np.float32)
    valid = np.ones(k, dtype=bool)
    return keypoints, sel_scores, d, valid


def kernel(features, orig_w, orig_h, max_keypoints, window):
    from concourse.bass_utils import run_bass_kernel_spmd

    features = np.asarray(features, dtype=np.float32)
    orig_w = int(orig_w)
    orig_h = int(orig_h)
    k = int(max_keypoints)
    window = int(window)

    assert features.shape == (1, D + 1, H, W), features.shape
    assert window == 5, f"kernel specialized for window=5, got {window}"
    assert k <= TOPC * H, k

    key = window
    if key not in _CACHE:
        _CACHE[key] = _build_module(window)
    nc = _CACHE[key]

    in_maps = make_in_maps(features, window)
    res = run_bass_kernel_spmd(nc, in_maps, core_ids=list(range(M)))
    return merge_outputs(res.results, orig_w, orig_h, k)
